# revision 4
# baseline (speedup 1.0000x reference)
"""Trainium2 Bass kernel for nn_DeformSpaceAttention (deformable 3x3 unfold +
per-channel max over taps + 1x1 conv + sigmoid).

V5 strategy (8 cores, data parallel over (batch b, H-half)):
  - Host precomputes per core: a zero-padded channels-last fp16 table
    tab[(ly)*116 + lx, c] covering the 66 rows the shard can touch, i16
    gather indices for the 18 (tap, y-corner) row-pair fetches of every
    output pixel, and the 4 bilinear corner weights per (pixel, tap).
  - Device, per (tap, 640-px chunk): SWDGE dma_gather (transpose=False)
    lands, for each pixel, the fp16 row-pair tab[idx, 0:512] (both
    x-neighbors x 256 channels) in pixel-partition layout [128px, 512].
  - 4 TensorScalarPtr products (4x DVE mode) scale each corner's 256
    channels by its per-pixel weight; 8 PE matmuls against a constant
    fp16 identity transpose-accumulate the 4 corners into PSUM in
    channel-partition layout [128c, px]; ACT pulls PSUM->SBUF fp16 and
    DVE keeps a running max over the 9 taps.
  - 1x1 conv = PE matmul with w0; sigmoid(+bias) on ACT; store.
"""

import sys
from contextlib import ExitStack

import numpy as np

for _p in ("/opt/pypackages", "/opt/trn_rl_repo"):
    if _p not in sys.path:
        sys.path.append(_p)

import concourse.bass as bass
import concourse.bacc as bacc
import concourse.mybir as mybir
from concourse.bass_utils import run_bass_kernel_spmd
from concourse.masks import make_identity
from concourse.tile import TileContext

F32 = mybir.dt.float32
F16 = mybir.dt.float16
I16 = mybir.dt.int16
ALU = mybir.AluOpType
ACTF = mybir.ActivationFunctionType

# ---- problem geometry (hardcoded; see spec) --------------------------------
B, C, H, W = 4, 256, 100, 100
PAD = 8                      # absorbs |offset| + kernel reach (<= 7.42)
N_CORES = 8
HALVES = N_CORES // B        # 2 shards per sample
RS = H // HALVES             # 50 rows per shard
TW = W + 2 * PAD             # table row width (116)
TR = RS + 2 * PAD            # table rows per shard (66)
NROWS = TR * TW              # 7656
NPX = RS * W                 # 5000 real pixels per shard
NPXP = 5120                  # padded to 40 x 128
NBLK = NPXP // 128           # 40
NCALL = 8                    # gather calls per (tap, y-corner)
GIDX = NPXP // NCALL         # 640 indices per gather call
GBLK = GIDX // 128           # 5 px-blocks per gather call
NSEG = NPXP // 512           # 10 psum groups (4 blocks each) per tap

KH = (np.arange(9) // 3 - 1).astype(np.float32)
KW = (np.arange(9) % 3 - 1).astype(np.float32)


def build_nc():
    nc = bacc.Bacc("TRN2", target_bir_lowering=False, debug=False,
                   num_swdge_queues=4)

    tab = nc.dram_tensor("tab", [NROWS, C], F16, kind="ExternalInput")
    idxT = nc.dram_tensor("idx", [128, 18, NPXP // 16], I16,
                          kind="ExternalInput")
    mapsT = nc.dram_tensor("maps", [128, NBLK, 36], F32, kind="ExternalInput")
    w0T = nc.dram_tensor("w0t", [128, 2], F32, kind="ExternalInput")
    b0s = nc.dram_tensor("b0s", [1, 1], F32, kind="ExternalInput")
    outd = nc.dram_tensor("out", [NPXP], F32, kind="ExternalOutput")

    # overlapping row-pair window: idx -> 512 contiguous fp16 (x0 and x0+1)
    tab_pairs = bass.AP(tensor=tab.ap().tensor, offset=0,
                        ap=[[C, NROWS - 1], [1, 2 * C]])

    with ExitStack() as ctx, TileContext(nc) as tc:
        with tc.tile_pool(name="const", bufs=1) as pconst:
            identh = pconst.tile([128, 128], F16, name="identh")
            make_identity(nc, identh[:])
            w0sb = pconst.tile([128, 2], F32, name="w0sb")
            nc.sync.dma_start(out=w0sb[:], in_=w0T.ap())
            w0h = pconst.tile([128, 2], F16, name="w0h")
            nc.vector.tensor_copy(out=w0h[:], in_=w0sb[:])
            b0sb = pconst.tile([1, 1], F32, name="b0sb")
            nc.sync.dma_start(out=b0sb[:], in_=b0s.ap())
            idxsb = pconst.tile([128, 18, NPXP // 16], I16, name="idxsb")
            nc.sync.dma_start(out=idxsb[:], in_=idxT.ap())
            mapssb = pconst.tile([128, NBLK, 36], F32, name="mapssb")
            nc.sync.dma_start(out=mapssb[:], in_=mapsT.ap())
            mapsflat = mapssb[:].rearrange("p b k -> p (b k)")
            # running per-channel max over taps, [128c, (cgrp, px)]
            acc = pconst.tile([128, 2, NPXP], F16, name="acc")

            with tc.tile_pool(name="pg", bufs=4) as pg, \
                 tc.tile_pool(name="pk", bufs=6) as pkp, \
                 tc.tile_pool(name="pps", bufs=6, space="PSUM") as pps, \
                 tc.tile_pool(name="psm", bufs=4) as psm, \
                 tc.tile_pool(name="pcv", bufs=2, space="PSUM") as pcv, \
                 tc.tile_pool(name="po", bufs=2) as po:
                for t in range(9):
                    ps = {}          # live psum tiles keyed by (seg, cg)
                    for chk in range(NCALL):
                        g = []
                        for a in range(2):
                            ga = pg.tile([128, GBLK, 2 * C], F16,
                                         name=f"g{a}")
                            c0 = chk * (GIDX // 16)
                            nc.gpsimd.dma_gather(
                                ga[:], tab_pairs,
                                idxsb[:][:, 2 * t + a, c0:c0 + GIDX // 16],
                                GIDX, GIDX, 2 * C, elem_step=C,
                                transpose=False,
                                queue_num=(2 * t + a) % 4)
                            g.append(ga)
                        for b5 in range(GBLK):
                            blk = chk * GBLK + b5
                            seg, pos = blk // 4, blk % 4
                            pk = pkp.tile([128, 4, C], F16, name="pk")
                            for k in range(4):
                                a, xc = k // 2, k % 2
                                nc.vector.tensor_scalar(
                                    pk[:][:, k],
                                    g[a][:][:, b5, xc * C:(xc + 1) * C],
                                    mapsflat[:, blk * 36 + 4 * t + k:
                                             blk * 36 + 4 * t + k + 1],
                                    None, ALU.mult)
                            if pos == 0:
                                for cg in range(2):
                                    ps[(seg, cg)] = pps.tile(
                                        [128, 512], F32, name="ps",
                                        space="PSUM")
                            for k in range(4):
                                for cg in range(2):
                                    nc.tensor.matmul(
                                        ps[(seg, cg)][:][:, pos * 128:
                                                         (pos + 1) * 128],
                                        pk[:][:, k, cg * 128:(cg + 1) * 128],
                                        identh[:],
                                        start=(k == 0), stop=(k == 3))
                            if pos == 3:
                                for cg in range(2):
                                    dst = acc[:][:, cg,
                                                 seg * 512:(seg + 1) * 512]
                                    if t == 0:
                                        nc.scalar.activation(
                                            out=dst, in_=ps[(seg, cg)][:],
                                            func=ACTF.Copy)
                                    else:
                                        smp = psm.tile([128, 512], F16,
                                                       name="smp")
                                        nc.scalar.activation(
                                            out=smp[:], in_=ps[(seg, cg)][:],
                                            func=ACTF.Copy)
                                        nc.vector.tensor_tensor(
                                            dst, dst, smp[:], ALU.max)
                                    del ps[(seg, cg)]

                # ---- 1x1 conv + sigmoid + store ------------------------
                osb = po.tile([1, NPXP], F32, name="osb")
                for seg in range(NSEG):
                    pc = pcv.tile([1, 512], F32, name="pc", space="PSUM")
                    for cg in range(2):
                        nc.tensor.matmul(
                            pc[:], w0h[:][:, cg:cg + 1],
                            acc[:][:, cg, seg * 512:(seg + 1) * 512],
                            start=(cg == 0), stop=(cg == 1))
                    nc.scalar.activation(
                        out=osb[:, seg * 512:(seg + 1) * 512], in_=pc[:],
                        func=ACTF.Sigmoid, bias=b0sb[:], scale=1.0)
                nc.sync.dma_start(out=outd.ap(), in_=osb[:])
    nc.compile()
    return nc


def host_prep(x, offset):
    """Per-core inputs. Core = b * HALVES + half."""
    in_maps = []
    ii = np.arange(NPXP)
    iic = np.minimum(ii, NPX - 1)
    for b in range(B):
        xp = np.zeros((H + 2 * PAD, W + 2 * PAD, C), np.float16)
        xp[PAD:PAD + H, PAD:PAD + W, :] = x[b].transpose(1, 2, 0)
        off = offset[b]                          # [18, H, W] f32
        for half in range(HALVES):
            h0 = half * RS
            tabh = np.ascontiguousarray(
                xp[h0:h0 + TR].reshape(NROWS, C))
            hh = iic // W + h0                   # global row of pixel
            ww = iic % W
            offpx = off[:, hh, ww].astype(np.float64)   # [18, NPXP]
            dy = offpx[0::2]
            dx = offpx[1::2]                     # [9, NPXP]
            py = hh[None] + KH[:, None] + dy
            px = ww[None] + KW[:, None] + dx
            y0 = np.floor(py)
            x0 = np.floor(px)
            wy = (py - y0).astype(np.float32)
            wx = (px - x0).astype(np.float32)
            ly = np.clip(y0 - (h0 - PAD), 0, TR - 2).astype(np.int64)
            lx = np.clip(x0 + PAD, 0, TW - 2).astype(np.int64)
            idx0 = ly * TW + lx                  # (y0) row-pair base
            idx1 = (ly + 1) * TW + lx            # (y0+1) row-pair base
            uy = 1.0 - wy
            ux = 1.0 - wx
            m = np.stack([uy * ux, uy * wx, wy * ux, wy * wx],
                         axis=1).astype(np.float32)      # [9, 4, NPXP]
            m[:, :, NPX:] = 0.0
            maps = np.ascontiguousarray(
                m.reshape(36, NPXP).T.reshape(NBLK, 128, 36)
                .transpose(1, 0, 2))             # [128, NBLK, 36]
            idxw = np.zeros((128, 18, NPXP // 16), np.int16)
            for t in range(9):
                for a in range(2):
                    src = (idx0 if a == 0 else idx1)[t].astype(np.int16)
                    wrap = src.reshape(NPXP // 16, 16).T     # [16, npx/16]
                    idxw[:, 2 * t + a, :] = np.tile(wrap, (8, 1))
            in_maps.append({"tab": tabh, "idx": idxw, "maps": maps})
    return in_maps


_NC_CACHE = {}


def get_nc():
    if "nc" not in _NC_CACHE:
        _NC_CACHE["nc"] = build_nc()
    return _NC_CACHE["nc"]


def kernel(x, offset, w0, b0, trace=False):
    x = np.asarray(x, np.float32)
    offset = np.asarray(offset, np.float32)
    w0 = np.asarray(w0, np.float32)
    b0 = np.asarray(b0, np.float32)
    nc = get_nc()
    in_maps = host_prep(x, offset)
    w0t = np.ascontiguousarray(w0.reshape(2, 128).T)
    for mm in in_maps:
        mm["w0t"] = w0t
        mm["b0s"] = b0.reshape(1, 1).astype(np.float32)
    if trace:
        try:
            import antenv.axon_hooks  # noqa: F401
        except ImportError:
            trace = False
    res = run_bass_kernel_spmd(nc, in_maps, core_ids=list(range(N_CORES)),
                               trace=trace)
    out = np.zeros((B, 1, H, W), np.float32)
    for core in range(N_CORES):
        b = core // HALVES
        half = core % HALVES
        h0 = half * RS
        o = res.results[core]["out"][:NPX].reshape(RS, W)
        out[b, 0, h0:h0 + RS] = o
    if trace:
        kernel.last_results = res
    return out


# revision 29
# speedup vs baseline: 5860.9690x; 5860.9690x over previous
"""Trainium2 Bass kernel for nn_DeformSpaceAttention (deformable 3x3 unfold +
per-channel max over taps + 1x1 conv + sigmoid).

V5 strategy (8 cores, data parallel over (batch b, H-half)):
  - Host precomputes per core: a zero-padded channels-last fp16 table
    tab[(ly)*116 + lx, c] covering the 66 rows the shard can touch, i16
    gather indices for the 18 (tap, y-corner) row-pair fetches of every
    output pixel, and the 4 bilinear corner weights per (pixel, tap).
  - Device, per (tap, 640-px chunk): SWDGE dma_gather (transpose=False)
    lands, for each pixel, the fp16 row-pair tab[idx, 0:512] (both
    x-neighbors x 256 channels) in pixel-partition layout [128px, 512].
  - 4 TensorScalarPtr products (4x DVE mode) scale each corner's 256
    channels by its per-pixel weight; 8 PE matmuls against a constant
    fp16 identity transpose-accumulate the 4 corners into PSUM in
    channel-partition layout [128c, px]; ACT pulls PSUM->SBUF fp16 and
    DVE keeps a running max over the 9 taps.
  - 1x1 conv = PE matmul with w0; sigmoid(+bias) on ACT; store.
"""

import sys
from contextlib import ExitStack

import numpy as np

for _p in ("/opt/pypackages", "/opt/trn_rl_repo"):
    if _p not in sys.path:
        sys.path.append(_p)

import concourse.bass as bass
import concourse.bacc as bacc
import concourse.mybir as mybir
from concourse.bass_utils import run_bass_kernel_spmd
from concourse.masks import make_identity
from concourse.tile import TileContext

F32 = mybir.dt.float32
F16 = mybir.dt.float16
I16 = mybir.dt.int16
ALU = mybir.AluOpType
ACTF = mybir.ActivationFunctionType

# ---- problem geometry (hardcoded; see spec) --------------------------------
B, C, H, W = 4, 256, 100, 100
PAD = 8                      # absorbs |offset| + kernel reach (<= 7.42)
N_CORES = 8
HALVES = N_CORES // B        # 2 shards per sample
RS = H // HALVES             # 50 rows per shard
TW = W + 2 * PAD             # table row width (116)
TR = RS + 2 * PAD            # table rows per shard (66)
NROWS = TR * TW              # 7656
NPX = RS * W                 # 5000 real pixels per shard
NPXP = 5120                  # padded to 40 x 128
NBLK = NPXP // 128           # 40
NCALL = 8                    # gather calls per (tap, y-corner)
GIDX = NPXP // NCALL         # 640 indices per gather call
GBLK = GIDX // 128           # 5 px-blocks per gather call
NSEG = NPXP // 512           # 10 psum groups (4 blocks each) per tap

KH = (np.arange(9) // 3 - 1).astype(np.float32)
KW = (np.arange(9) % 3 - 1).astype(np.float32)


def build_nc():
    nc = bacc.Bacc("TRN2", target_bir_lowering=False, debug=False,
                   num_swdge_queues=4)

    tab = nc.dram_tensor("tab", [NROWS, C], F16, kind="ExternalInput")
    idxT = nc.dram_tensor("idx", [128, 18, NPXP // 16], I16,
                          kind="ExternalInput")
    mapsT = nc.dram_tensor("maps", [128, NBLK, 36], F32, kind="ExternalInput")
    w0T = nc.dram_tensor("w0t", [128, 2], F32, kind="ExternalInput")
    b0s = nc.dram_tensor("b0s", [1, 1], F32, kind="ExternalInput")
    outd = nc.dram_tensor("out", [NPXP], F32, kind="ExternalOutput")

    # overlapping row-pair window: idx -> 512 contiguous fp16 (x0 and x0+1)
    tab_pairs = bass.AP(tensor=tab.ap().tensor, offset=0,
                        ap=[[C, NROWS - 1], [1, 2 * C]])

    with ExitStack() as ctx, TileContext(nc) as tc:
        with tc.tile_pool(name="const", bufs=1) as pconst:
            identh = pconst.tile([128, 128], F16, name="identh")
            make_identity(nc, identh[:])
            w0sb = pconst.tile([128, 2], F32, name="w0sb")
            nc.sync.dma_start(out=w0sb[:], in_=w0T.ap())
            w0h = pconst.tile([128, 2], F16, name="w0h")
            nc.vector.tensor_copy(out=w0h[:], in_=w0sb[:])
            b0sb = pconst.tile([1, 1], F32, name="b0sb")
            nc.sync.dma_start(out=b0sb[:], in_=b0s.ap())
            idxsb = pconst.tile([128, 18, NPXP // 16], I16, name="idxsb")
            # split load: first two (tap, a) columns unblock tap-0 gathers
            nc.sync.dma_start(out=idxsb[:, 0:2], in_=idxT.ap()[:, 0:2])
            mapssb = pconst.tile([128, NBLK, 36], F32, name="mapssb")
            nc.sync.dma_start(out=mapssb[:], in_=mapsT.ap())
            nc.sync.dma_start(out=idxsb[:, 2:18], in_=idxT.ap()[:, 2:18])
            mapsflat = mapssb[:].rearrange("p b k -> p (b k)")
            # running per-channel max over taps, [128c, (cgrp, px)]
            acc = pconst.tile([128, 2, NPXP], F16, name="acc")

            with tc.tile_pool(name="pg", bufs=6) as pg, \
                 tc.tile_pool(name="pk", bufs=6) as pkp, \
                 tc.tile_pool(name="pps", bufs=6, space="PSUM") as pps, \
                 tc.tile_pool(name="psm", bufs=2) as psm, \
                 tc.tile_pool(name="pcv", bufs=2, space="PSUM") as pcv, \
                 tc.tile_pool(name="po", bufs=2) as po:
                def conv_phase(ph, SPP):
                    for sp in range(SPP):
                        seg = ph * SPP + sp
                        pc = pcv.tile([1, 512], F32, name="pc", space="PSUM")
                        for cg in range(2):
                            nc.tensor.matmul(
                                pc[:], w0h[:][:, cg:cg + 1],
                                acc[:][:, cg, seg * 512:(seg + 1) * 512],
                                start=(cg == 0), stop=(cg == 1))
                        osb = po.tile([1, 512], F32, name="osb")
                        nc.scalar.activation(
                            out=osb[:], in_=pc[:],
                            func=ACTF.Sigmoid, bias=b0sb[:], scale=1.0)
                        nc.sync.dma_start(
                            out=outd.ap()[seg * 512:(seg + 1) * 512],
                            in_=osb[:])

                pending = []
                samp = {}

                def pull_group(ft, fseg, fps):
                    # taps 1..7: ACT pulls PSUM into the per-tap staging
                    # buffer (tap 0: straight into acc); one wide DVE max
                    # per tap is deferred via `pending`. Tap 8 (the tail):
                    # DVE maxes straight from PSUM and the segment's conv
                    # + sigmoid + store chain fires immediately.
                    if ft >= 7:
                        for cg in range(2):
                            dst = acc[:][:, cg,
                                         fseg * 512:(fseg + 1) * 512]
                            nc.scalar.activation(
                                out=samp[ft][:][:, cg,
                                               fseg * 512:(fseg + 1) * 512],
                                in_=fps[cg][:], func=ACTF.Copy)
                            nc.vector.tensor_tensor(
                                dst, dst,
                                samp[ft][:][:, cg,
                                           fseg * 512:(fseg + 1) * 512],
                                ALU.max)
                        if ft == 8:
                            conv_phase(fseg, 1)
                        return
                    for cg in range(2):
                        if ft == 0:
                            nc.scalar.activation(
                                out=acc[:][:, cg,
                                           fseg * 512:(fseg + 1) * 512],
                                in_=fps[cg][:], func=ACTF.Copy)
                        else:
                            nc.scalar.activation(
                                out=samp[ft][:][:, cg,
                                               fseg * 512:(fseg + 1) * 512],
                                in_=fps[cg][:], func=ACTF.Copy)

                def flush_pending(limit):
                    while len(pending) > limit:
                        ft = pending.pop(0)
                        nc.vector.tensor_tensor(
                            acc[:], acc[:], samp.pop(ft)[:], ALU.max)

                for t in range(9):
                    ps = {}
                    if t > 0:
                        samp[t] = psm.tile([128, 2, NPXP], F16, name="samp")
                    for chk in range(NCALL):
                        g = []
                        for a in range(2):
                            ga = pg.tile([128, GBLK, 2 * C], F16,
                                         name=f"g{a}")
                            c0 = chk * (GIDX // 16)
                            nc.gpsimd.dma_gather(
                                ga[:], tab_pairs,
                                idxsb[:][:, 2 * t + a,
                                         c0:c0 + GIDX // 16],
                                GIDX, GIDX, 2 * C, elem_step=C,
                                transpose=False,
                                queue_num=(2 * t + a) % 4)
                            g.append(ga)
                        for b5 in range(GBLK):
                            blk = chk * GBLK + b5
                            seg, pos = blk // 4, blk % 4
                            pk = pkp.tile([128, 4, C], F16, name="pk")
                            for k in range(4):
                                a, xc = k // 2, k % 2
                                sc = mapsflat[:, blk * 36 + 4 * t + k:
                                              blk * 36 + 4 * t + k + 1]
                                gin = g[a][:][:, b5, xc * C:(xc + 1) * C]
                                if k == 3 and b5 % 2 == 1:
                                    # offload some products to ACT (slack)
                                    nc.scalar.activation(
                                        out=pk[:][:, k], in_=gin,
                                        func=ACTF.Copy, scale=sc)
                                else:
                                    nc.vector.tensor_scalar(
                                        pk[:][:, k], gin, sc,
                                        None, ALU.mult)
                            if pos == 0:
                                ps[seg] = [pps.tile([128, 512], F32,
                                                    name="ps", space="PSUM")
                                           for _ in range(2)]
                            for k in range(4):
                                for cg in range(2):
                                    nc.tensor.matmul(
                                        ps[seg][cg][:][:, pos * 128:
                                                       (pos + 1) * 128],
                                        pk[:][:, k,
                                              cg * 128:(cg + 1) * 128],
                                        identh[:],
                                        start=(k == 0), stop=(k == 3))
                            if pos == 3:
                                pull_group(t, seg, ps.pop(seg))
                    if 0 < t < 7:
                        pending.append(t)
                        flush_pending(1)
                    if t == 6:
                        flush_pending(0)
    nc.compile()
    return nc


def host_prep(x, offset):
    """Per-core inputs. Core = b * HALVES + half."""
    in_maps = []
    ii = np.arange(NPXP)
    iic = np.minimum(ii, NPX - 1)
    for b in range(B):
        xp = np.zeros((H + 2 * PAD, W + 2 * PAD, C), np.float16)
        xp[PAD:PAD + H, PAD:PAD + W, :] = x[b].transpose(1, 2, 0)
        off = offset[b]                          # [18, H, W] f32
        for half in range(HALVES):
            h0 = half * RS
            tabh = np.ascontiguousarray(
                xp[h0:h0 + TR].reshape(NROWS, C))
            hh = iic // W + h0                   # global row of pixel
            ww = iic % W
            offpx = off[:, hh, ww].astype(np.float64)   # [18, NPXP]
            dy = offpx[0::2]
            dx = offpx[1::2]                     # [9, NPXP]
            py = hh[None] + KH[:, None] + dy
            px = ww[None] + KW[:, None] + dx
            y0 = np.floor(py)
            x0 = np.floor(px)
            wy = (py - y0).astype(np.float32)
            wx = (px - x0).astype(np.float32)
            ly = np.clip(y0 - (h0 - PAD), 0, TR - 2).astype(np.int64)
            lx = np.clip(x0 + PAD, 0, TW - 2).astype(np.int64)
            idx0 = ly * TW + lx                  # (y0) row-pair base
            idx1 = (ly + 1) * TW + lx            # (y0+1) row-pair base
            uy = 1.0 - wy
            ux = 1.0 - wx
            m = np.stack([uy * ux, uy * wx, wy * ux, wy * wx],
                         axis=1).astype(np.float32)      # [9, 4, NPXP]
            m[:, :, NPX:] = 0.0
            maps = np.ascontiguousarray(
                m.reshape(36, NPXP).T.reshape(NBLK, 128, 36)
                .transpose(1, 0, 2))             # [128, NBLK, 36]
            idxw = np.zeros((128, 18, NPXP // 16), np.int16)
            for t in range(9):
                for a in range(2):
                    src = (idx0 if a == 0 else idx1)[t].astype(np.int16)
                    wrap = src.reshape(NPXP // 16, 16).T     # [16, npx/16]
                    idxw[:, 2 * t + a, :] = np.tile(wrap, (8, 1))
            in_maps.append({"tab": tabh, "idx": idxw, "maps": maps})
    return in_maps


_NC_CACHE = {}


def get_nc():
    if "nc" not in _NC_CACHE:
        _NC_CACHE["nc"] = build_nc()
    return _NC_CACHE["nc"]


def kernel(x, offset, w0, b0, trace=False):
    x = np.asarray(x, np.float32)
    offset = np.asarray(offset, np.float32)
    w0 = np.asarray(w0, np.float32)
    b0 = np.asarray(b0, np.float32)
    nc = get_nc()
    in_maps = host_prep(x, offset)
    w0t = np.ascontiguousarray(w0.reshape(2, 128).T)
    for mm in in_maps:
        mm["w0t"] = w0t
        mm["b0s"] = b0.reshape(1, 1).astype(np.float32)
    if trace:
        try:
            import antenv.axon_hooks  # noqa: F401
        except ImportError:
            trace = False
    res = run_bass_kernel_spmd(nc, in_maps, core_ids=list(range(N_CORES)),
                               trace=trace)
    out = np.zeros((B, 1, H, W), np.float32)
    for core in range(N_CORES):
        b = core // HALVES
        half = core % HALVES
        h0 = half * RS
        o = res.results[core]["out"][:NPX].reshape(RS, W)
        out[b, 0, h0:h0 + RS] = o
    if trace:
        kernel.last_results = res
    return out


# revision 35
# speedup vs baseline: 5882.0113x; 1.0036x over previous
"""Trainium2 Bass kernel for nn_DeformSpaceAttention (deformable 3x3 unfold +
per-channel max over taps + 1x1 conv + sigmoid).

V5 strategy (8 cores, data parallel over (batch b, H-half)):
  - Host precomputes per core: a zero-padded channels-last fp16 table
    tab[(ly)*116 + lx, c] covering the 66 rows the shard can touch, i16
    gather indices for the 18 (tap, y-corner) row-pair fetches of every
    output pixel, and the 4 bilinear corner weights per (pixel, tap).
  - Device, per (tap, 640-px chunk): SWDGE dma_gather (transpose=False)
    lands, for each pixel, the fp16 row-pair tab[idx, 0:512] (both
    x-neighbors x 256 channels) in pixel-partition layout [128px, 512].
  - 4 TensorScalarPtr products (4x DVE mode) scale each corner's 256
    channels by its per-pixel weight; 8 PE matmuls against a constant
    fp16 identity transpose-accumulate the 4 corners into PSUM in
    channel-partition layout [128c, px]; ACT pulls PSUM->SBUF fp16 and
    DVE keeps a running max over the 9 taps.
  - 1x1 conv = PE matmul with w0; sigmoid(+bias) on ACT; store.
"""

import sys
from contextlib import ExitStack

import numpy as np

for _p in ("/opt/pypackages", "/opt/trn_rl_repo"):
    if _p not in sys.path:
        sys.path.append(_p)

import concourse.bass as bass
import concourse.bacc as bacc
import concourse.mybir as mybir
from concourse.bass_utils import run_bass_kernel_spmd
from concourse.masks import make_identity
from concourse.tile import TileContext

F32 = mybir.dt.float32
F16 = mybir.dt.float16
I16 = mybir.dt.int16
ALU = mybir.AluOpType
ACTF = mybir.ActivationFunctionType

# ---- problem geometry (hardcoded; see spec) --------------------------------
B, C, H, W = 4, 256, 100, 100
PAD = 8                      # absorbs |offset| + kernel reach (<= 7.42)
N_CORES = 8
HALVES = N_CORES // B        # 2 shards per sample
RS = H // HALVES             # 50 rows per shard
TW = W + 2 * PAD             # table row width (116)
TR = RS + 2 * PAD            # table rows per shard (66)
NROWS = TR * TW              # 7656
NPX = RS * W                 # 5000 real pixels per shard
NPXP = 5120                  # padded to 40 x 128
NBLK = NPXP // 128           # 40
NCALL = 8                    # gather calls per (tap, y-corner)
GIDX = NPXP // NCALL         # 640 indices per gather call
GBLK = GIDX // 128           # 5 px-blocks per gather call
NSEG = NPXP // 512           # 10 psum groups (4 blocks each) per tap

KH = (np.arange(9) // 3 - 1).astype(np.float32)
KW = (np.arange(9) % 3 - 1).astype(np.float32)


def build_nc():
    nc = bacc.Bacc("TRN2", target_bir_lowering=False, debug=False,
                   num_swdge_queues=4)

    tab = nc.dram_tensor("tab", [NROWS, C], F16, kind="ExternalInput")
    idxT = nc.dram_tensor("idx", [128, 18, NPXP // 16], I16,
                          kind="ExternalInput")
    mapsT = nc.dram_tensor("maps", [128, NBLK, 36], F32, kind="ExternalInput")
    w0T = nc.dram_tensor("w0t", [128, 2], F32, kind="ExternalInput")
    b0s = nc.dram_tensor("b0s", [1, 1], F32, kind="ExternalInput")
    outd = nc.dram_tensor("out", [NPXP], F32, kind="ExternalOutput")

    # overlapping row-pair window: idx -> 512 contiguous fp16 (x0 and x0+1)
    tab_pairs = bass.AP(tensor=tab.ap().tensor, offset=0,
                        ap=[[C, NROWS - 1], [1, 2 * C]])

    with ExitStack() as ctx, TileContext(nc) as tc:
        with tc.tile_pool(name="const", bufs=1) as pconst:
            identh = pconst.tile([128, 128], F16, name="identh")
            make_identity(nc, identh[:])
            w0sb = pconst.tile([128, 2], F32, name="w0sb")
            nc.sync.dma_start(out=w0sb[:], in_=w0T.ap())
            w0h = pconst.tile([128, 2], F16, name="w0h")
            nc.vector.tensor_copy(out=w0h[:], in_=w0sb[:])
            b0sb = pconst.tile([1, 1], F32, name="b0sb")
            nc.sync.dma_start(out=b0sb[:], in_=b0s.ap())
            idxsb = pconst.tile([128, 18, NPXP // 16], I16, name="idxsb")
            # split load: first two (tap, a) columns unblock tap-0 gathers
            nc.sync.dma_start(out=idxsb[:, 0:2], in_=idxT.ap()[:, 0:2])
            mapssb = pconst.tile([128, NBLK, 36], F32, name="mapssb")
            nc.sync.dma_start(out=mapssb[:], in_=mapsT.ap())
            nc.sync.dma_start(out=idxsb[:, 2:18], in_=idxT.ap()[:, 2:18])
            mapsflat = mapssb[:].rearrange("p b k -> p (b k)")
            # running per-channel max over taps, [128c, (cgrp, px)]
            acc = pconst.tile([128, 2, NPXP], F16, name="acc")

            with tc.tile_pool(name="pg", bufs=6) as pg, \
                 tc.tile_pool(name="pk", bufs=6) as pkp, \
                 tc.tile_pool(name="pps", bufs=6, space="PSUM") as pps, \
                 tc.tile_pool(name="psm", bufs=2) as psm, \
                 tc.tile_pool(name="pcv", bufs=2, space="PSUM") as pcv, \
                 tc.tile_pool(name="po", bufs=2) as po:
                def conv_phase(ph, SPP):
                    for sp in range(SPP):
                        seg = ph * SPP + sp
                        pc = pcv.tile([1, 512], F32, name="pc", space="PSUM")
                        for cg in range(2):
                            nc.tensor.matmul(
                                pc[:], w0h[:][:, cg:cg + 1],
                                acc[:][:, cg, seg * 512:(seg + 1) * 512],
                                start=(cg == 0), stop=(cg == 1))
                        osb = po.tile([1, 512], F32, name="osb")
                        nc.scalar.activation(
                            out=osb[:], in_=pc[:],
                            func=ACTF.Sigmoid, bias=b0sb[:], scale=1.0)
                        nc.sync.dma_start(
                            out=outd.ap()[seg * 512:(seg + 1) * 512],
                            in_=osb[:])

                pending = []
                samp = {}

                def pull_group(ft, fseg, fps):
                    # taps 1..7: ACT pulls PSUM into the per-tap staging
                    # buffer (tap 0: straight into acc); one wide DVE max
                    # per tap is deferred via `pending`. Tap 8 (the tail):
                    # DVE maxes straight from PSUM and the segment's conv
                    # + sigmoid + store chain fires immediately.
                    if ft == 8:
                        for cg in range(2):
                            dst = acc[:][:, cg,
                                         fseg * 512:(fseg + 1) * 512]
                            nc.scalar.activation(
                                out=samp[8][:][:, cg,
                                               fseg * 512:(fseg + 1) * 512],
                                in_=fps[cg][:], func=ACTF.Copy)
                            nc.vector.tensor_tensor(
                                dst, dst,
                                samp[8][:][:, cg,
                                           fseg * 512:(fseg + 1) * 512],
                                ALU.max)
                        conv_phase(fseg, 1)
                        return
                    for cg in range(2):
                        if ft == 0:
                            nc.scalar.activation(
                                out=acc[:][:, cg,
                                           fseg * 512:(fseg + 1) * 512],
                                in_=fps[cg][:], func=ACTF.Copy)
                        else:
                            nc.scalar.activation(
                                out=samp[ft][:][:, cg,
                                               fseg * 512:(fseg + 1) * 512],
                                in_=fps[cg][:], func=ACTF.Copy)

                def flush_pending(limit):
                    while len(pending) > limit:
                        ft = pending.pop(0)
                        nc.vector.tensor_tensor(
                            acc[:], acc[:], samp.pop(ft)[:], ALU.max)

                def do_blocks(t, g, blk0, nblk, bideo, ps):
                    for b5 in range(nblk):
                        blk = blk0 + b5
                        seg, pos = blk // 4, blk % 4
                        pk = pkp.tile([128, 4, C], F16, name="pk")
                        for k in range(4):
                            a, xc = k // 2, k % 2
                            sc = mapsflat[:, blk * 36 + 4 * t + k:
                                          blk * 36 + 4 * t + k + 1]
                            gin = g[a][:][:, b5, xc * C:(xc + 1) * C]
                            if k == 3 and (blk0 + b5) % 2 == bideo:
                                # offload some products to ACT (slack)
                                nc.scalar.activation(
                                    out=pk[:][:, k], in_=gin,
                                    func=ACTF.Copy, scale=sc)
                            else:
                                nc.vector.tensor_scalar(
                                    pk[:][:, k], gin, sc,
                                    None, ALU.mult)
                        if pos == 0:
                            ps[seg] = [pps.tile([128, 512], F32,
                                                name="ps", space="PSUM")
                                       for _ in range(2)]
                        for k in range(4):
                            for cg in range(2):
                                nc.tensor.matmul(
                                    ps[seg][cg][:][:, pos * 128:
                                                   (pos + 1) * 128],
                                    pk[:][:, k,
                                          cg * 128:(cg + 1) * 128],
                                    identh[:],
                                    start=(k == 0), stop=(k == 3))
                        if pos == 3:
                            pull_group(t, seg, ps.pop(seg))

                def gather_pair(t, c0, n):
                    g = []
                    for a in range(2):
                        ga = pg.tile([128, n // 128, 2 * C], F16,
                                     name=f"g{a}")
                        nc.gpsimd.dma_gather(
                            ga[:], tab_pairs,
                            idxsb[:][:, 2 * t + a,
                                     c0 // 16:(c0 + n) // 16],
                            n, n, 2 * C, elem_step=C,
                            transpose=False,
                            queue_num=(2 * t + a) % 4)
                        g.append(ga)
                    return g

                for t in range(7):
                    ps = {}
                    if t > 0:
                        samp[t] = psm.tile([128, 2, NPXP], F16, name="samp")
                    for chk in range(NCALL):
                        g = gather_pair(t, chk * GIDX, GIDX)
                        do_blocks(t, g, chk * GBLK, GBLK, 1, ps)
                    if t > 0:
                        pending.append(t)
                        flush_pending(1)
                flush_pending(0)

                # tap 7: normal wide-max pipeline
                ps = {}
                samp[7] = psm.tile([128, 2, NPXP], F16, name="samp")
                for chk in range(NCALL):
                    g = gather_pair(7, chk * GIDX, GIDX)
                    do_blocks(7, g, chk * GBLK, GBLK, 2, ps)
                pending.append(7)
                flush_pending(0)

                # endgame tap 8: per-seg max + conv chained per segment
                samp[8] = psm.tile([128, 2, NPXP], F16, name="samp")
                ps8 = {}
                for chk in range(NCALL):
                    g = gather_pair(8, chk * GIDX, GIDX)
                    do_blocks(8, g, chk * GBLK, GBLK, 2, ps8)
    nc.compile()
    return nc


def host_prep(x, offset):
    """Per-core inputs. Core = b * HALVES + half."""
    in_maps = []
    ii = np.arange(NPXP)
    iic = np.minimum(ii, NPX - 1)
    for b in range(B):
        xp = np.zeros((H + 2 * PAD, W + 2 * PAD, C), np.float16)
        xp[PAD:PAD + H, PAD:PAD + W, :] = x[b].transpose(1, 2, 0)
        off = offset[b]                          # [18, H, W] f32
        for half in range(HALVES):
            h0 = half * RS
            tabh = np.ascontiguousarray(
                xp[h0:h0 + TR].reshape(NROWS, C))
            hh = iic // W + h0                   # global row of pixel
            ww = iic % W
            offpx = off[:, hh, ww].astype(np.float64)   # [18, NPXP]
            dy = offpx[0::2]
            dx = offpx[1::2]                     # [9, NPXP]
            py = hh[None] + KH[:, None] + dy
            px = ww[None] + KW[:, None] + dx
            y0 = np.floor(py)
            x0 = np.floor(px)
            wy = (py - y0).astype(np.float32)
            wx = (px - x0).astype(np.float32)
            ly = np.clip(y0 - (h0 - PAD), 0, TR - 2).astype(np.int64)
            lx = np.clip(x0 + PAD, 0, TW - 2).astype(np.int64)
            idx0 = ly * TW + lx                  # (y0) row-pair base
            idx1 = (ly + 1) * TW + lx            # (y0+1) row-pair base
            uy = 1.0 - wy
            ux = 1.0 - wx
            m = np.stack([uy * ux, uy * wx, wy * ux, wy * wx],
                         axis=1).astype(np.float32)      # [9, 4, NPXP]
            m[:, :, NPX:] = 0.0
            maps = np.ascontiguousarray(
                m.reshape(36, NPXP).T.reshape(NBLK, 128, 36)
                .transpose(1, 0, 2))             # [128, NBLK, 36]
            idxw = np.zeros((128, 18, NPXP // 16), np.int16)
            for t in range(9):
                for a in range(2):
                    src = (idx0 if a == 0 else idx1)[t].astype(np.int16)
                    wrap = src.reshape(NPXP // 16, 16).T     # [16, npx/16]
                    idxw[:, 2 * t + a, :] = np.tile(wrap, (8, 1))
            in_maps.append({"tab": tabh, "idx": idxw, "maps": maps})
    return in_maps


_NC_CACHE = {}


def get_nc():
    if "nc" not in _NC_CACHE:
        _NC_CACHE["nc"] = build_nc()
    return _NC_CACHE["nc"]


def kernel(x, offset, w0, b0, trace=False):
    x = np.asarray(x, np.float32)
    offset = np.asarray(offset, np.float32)
    w0 = np.asarray(w0, np.float32)
    b0 = np.asarray(b0, np.float32)
    nc = get_nc()
    in_maps = host_prep(x, offset)
    w0t = np.ascontiguousarray(w0.reshape(2, 128).T)
    for mm in in_maps:
        mm["w0t"] = w0t
        mm["b0s"] = b0.reshape(1, 1).astype(np.float32)
    if trace:
        try:
            import antenv.axon_hooks  # noqa: F401
        except ImportError:
            trace = False
    res = run_bass_kernel_spmd(nc, in_maps, core_ids=list(range(N_CORES)),
                               trace=trace)
    out = np.zeros((B, 1, H, W), np.float32)
    for core in range(N_CORES):
        b = core // HALVES
        half = core % HALVES
        h0 = half * RS
        o = res.results[core]["out"][:NPX].reshape(RS, W)
        out[b, 0, h0:h0 + RS] = o
    if trace:
        kernel.last_results = res
    return out
